# revision 10
# baseline (speedup 1.0000x reference)
"""Trainium2 Bass kernel for nn_PositionalEncoding_61151744360729.

out[b, s, n, :] = x[b, s, n, :] + ||x[b, s+1, n, :] - x[b, s, n, :]||_2
(with distance 0 at s = S-1).

Sharding: data-parallel on batch across 8 NeuronCores (64 batches/core).
On-core layout: partition p = b*2 + h (b = batch, h = sequence half),
free dim = frames*75 floats, so every DMA is a large contiguous span per
partition and the outermost AP dim (64) lets SWDGE fan descriptors over
all 16 SDMA engines. Each batch is padded host-side with a copy of its
last frame, which makes the last-frame distance exactly 0 with no
special-casing. Per 64-frame chunk: DVE shifted subtract -> ACT square
-> two strided DVE adds (sum over the 3 coords) -> ACT sqrt -> three
strided DVE broadcast-adds -> DMA out.
"""

import sys
from contextlib import ExitStack

for _p in ("/opt/trn_rl_repo", "/root/.axon_site/_ro/trn_rl_repo"):
    if _p not in sys.path:
        sys.path.insert(0, _p)

import numpy as np

import concourse.bass as bass
import concourse.tile as tile
from concourse import bacc, mybir
from concourse.bass_utils import run_bass_kernel_spmd

B, S, N, C = 512, 1024, 25, 3
FW = N * C                 # 75 floats per frame
NCORES = 8
BC = B // NCORES           # 64 batches per core
H = 2                      # sequence halves -> 128 partitions
SH = S // H                # 512 frames per half
P = H * BC                 # 128 partitions
F = 64                     # frames per chunk per partition
K = SH // F                # 8 chunks
IN_FLAT = BC * (S + 1) * FW   # input padded by one zero frame per batch
OUT_FLAT = BC * S * FW

_cache = {}


def _build():
    f32 = mybir.dt.float32
    Af = mybir.ActivationFunctionType
    nc = bacc.Bacc(
        "TRN2", target_bir_lowering=False, debug=False, num_devices=NCORES
    )
    xin = nc.dram_tensor("xin", [IN_FLAT], f32, kind="ExternalInput")
    yout = nc.dram_tensor("yout", [OUT_FLAT], f32, kind="ExternalOutput")

    with tile.TileContext(nc) as tc, ExitStack() as ctx:
        pin = ctx.enter_context(tc.tile_pool(name="pin", bufs=3))
        pmid = ctx.enter_context(tc.tile_pool(name="pmid", bufs=3))
        psm = ctx.enter_context(tc.tile_pool(name="psm", bufs=2))
        pout = ctx.enter_context(tc.tile_pool(name="pout", bufs=2))

        for k in range(K):
            in_t = pin.tile([P, (F + 1) * FW], f32)
            src = bass.AP(
                xin,
                k * F * FW,
                [[(S + 1) * FW, BC], [SH * FW, H], [1, (F + 1) * FW]],
            )
            nc.gpsimd.dma_start(in_t[:], src)

            diff_t = pmid.tile([P, F * FW], f32)
            nc.vector.tensor_sub(
                diff_t[:], in_t[:, FW:(F + 1) * FW], in_t[:, 0:F * FW]
            )
            nc.scalar.activation(diff_t[:], diff_t[:], Af.Square)

            sq4 = diff_t[:].rearrange("p (f n c) -> p f n c", f=F, n=N, c=C)
            dist2_t = psm.tile([P, F * N], f32)
            d2 = dist2_t[:].rearrange("p (f n) -> p f n", f=F)
            nc.gpsimd.tensor_add(d2, sq4[:, :, :, 0], sq4[:, :, :, 1])
            nc.gpsimd.tensor_add(d2, d2, sq4[:, :, :, 2])
            dist_t = psm.tile([P, F * N], f32)
            nc.scalar.activation(dist_t[:], dist2_t[:], Af.Sqrt)

            out_t = pout.tile([P, F * FW], f32)
            out4 = out_t[:].rearrange("p (f n c) -> p f n c", f=F, n=N, c=C)
            in4 = in_t[:, 0:F * FW].rearrange(
                "p (f n c) -> p f n c", f=F, n=N, c=C
            )
            dvb = (
                dist_t[:]
                .rearrange("p (f n) -> p f n", f=F)
                .unsqueeze(3)
                .broadcast_to([P, F, N, C])
            )
            nc.vector.tensor_add(out4, in4, dvb)

            dst = bass.AP(
                yout,
                k * F * FW,
                [[S * FW, BC], [SH * FW, H], [1, F * FW]],
            )
            nc.gpsimd.dma_start(dst, out_t[:])

    nc.compile()
    return nc


def kernel(x: np.ndarray, **_unused) -> np.ndarray:
    x = np.ascontiguousarray(np.asarray(x), dtype=np.float32)
    assert x.shape == (B, S, N, C), x.shape

    if "nc" not in _cache:
        _cache["nc"] = _build()
    nc = _cache["nc"]

    in_maps = []
    for ci in range(NCORES):
        xc = x[ci * BC:(ci + 1) * BC].reshape(BC, S * FW)
        xp = np.empty((BC, (S + 1) * FW), dtype=np.float32)
        xp[:, : S * FW] = xc
        # pad frame = copy of the last frame -> diff at s = S-1 is 0
        xp[:, S * FW:] = xc[:, (S - 1) * FW:]
        in_maps.append({"xin": xp.reshape(IN_FLAT)})

    res = run_bass_kernel_spmd(nc, in_maps, core_ids=list(range(NCORES)))
    _cache["last_results"] = res

    out = np.empty((B, S, N, C), dtype=np.float32)
    for ci in range(NCORES):
        out[ci * BC:(ci + 1) * BC] = res.results[ci]["yout"].reshape(
            BC, S, N, C
        )
    return out


# revision 12
# speedup vs baseline: 1.1426x; 1.1426x over previous
"""Trainium2 Bass kernel for nn_PositionalEncoding_61151744360729.

out[b, s, n, :] = x[b, s, n, :] + ||x[b, s+1, n, :] - x[b, s, n, :]||_2
(with distance 0 at s = S-1).

Sharding: data-parallel on batch across 8 NeuronCores (64 batches/core).
On-core layout: partition p = b*2 + h (b = batch, h = sequence half),
free dim = frames*75 floats, so every DMA is a large contiguous span per
partition and the outermost AP dim (64) lets SWDGE fan descriptors over
all 16 SDMA engines. Each batch is padded host-side with a copy of its
last frame, which makes the last-frame distance exactly 0 with no
special-casing. Per 64-frame chunk: DVE shifted subtract -> ACT square
-> two strided DVE adds (sum over the 3 coords) -> ACT sqrt -> three
strided DVE broadcast-adds -> DMA out.
"""

import sys
from contextlib import ExitStack

for _p in ("/opt/trn_rl_repo", "/root/.axon_site/_ro/trn_rl_repo"):
    if _p not in sys.path:
        sys.path.insert(0, _p)

import numpy as np

import concourse.bass as bass
import concourse.tile as tile
from concourse import bacc, mybir
from concourse.bass_utils import run_bass_kernel_spmd

B, S, N, C = 512, 1024, 25, 3
FW = N * C                 # 75 floats per frame
NCORES = 8
BC = B // NCORES           # 64 batches per core
H = 2                      # sequence halves -> 128 partitions
SH = S // H                # 512 frames per half
P = H * BC                 # 128 partitions
F = 64                     # frames per chunk per partition
K = SH // F                # 8 chunks
IN_FLAT = BC * (S + 1) * FW   # input padded by one zero frame per batch
OUT_FLAT = BC * S * FW

_cache = {}


def _build():
    f32 = mybir.dt.float32
    Af = mybir.ActivationFunctionType
    nc = bacc.Bacc(
        "TRN2", target_bir_lowering=False, debug=False, num_devices=NCORES
    )
    xin = nc.dram_tensor("xin", [IN_FLAT], f32, kind="ExternalInput")
    yout = nc.dram_tensor("yout", [OUT_FLAT], f32, kind="ExternalOutput")

    with tile.TileContext(nc) as tc, ExitStack() as ctx:
        pin = ctx.enter_context(tc.tile_pool(name="pin", bufs=4))
        pmid = ctx.enter_context(tc.tile_pool(name="pmid", bufs=2))
        psm = ctx.enter_context(tc.tile_pool(name="psm", bufs=2))
        pout = ctx.enter_context(tc.tile_pool(name="pout", bufs=2))

        for k in range(K):
            in_t = pin.tile([P, (F + 1) * FW], f32)
            src = bass.AP(
                xin,
                k * F * FW,
                [[(S + 1) * FW, BC], [SH * FW, H], [1, (F + 1) * FW]],
            )
            nc.gpsimd.dma_start(in_t[:], src)

            diff_t = pmid.tile([P, F * FW], f32)
            nc.vector.tensor_sub(
                diff_t[:], in_t[:, FW:(F + 1) * FW], in_t[:, 0:F * FW]
            )
            nc.scalar.activation(diff_t[:], diff_t[:], Af.Square)

            sq4 = diff_t[:].rearrange("p (f n c) -> p f n c", f=F, n=N, c=C)
            dist2_t = psm.tile([P, F * N], f32)
            d2 = dist2_t[:].rearrange("p (f n) -> p f n", f=F)
            nc.gpsimd.tensor_add(d2, sq4[:, :, :, 0], sq4[:, :, :, 1])
            nc.vector.tensor_add(d2, d2, sq4[:, :, :, 2])
            # sqrt in place: dist2_t becomes dist
            nc.scalar.activation(dist2_t[:], dist2_t[:], Af.Sqrt)
            dist_t = dist2_t

            out_t = pout.tile([P, F * FW], f32)
            out4 = out_t[:].rearrange("p (f n c) -> p f n c", f=F, n=N, c=C)
            in4 = in_t[:, 0:F * FW].rearrange(
                "p (f n c) -> p f n c", f=F, n=N, c=C
            )
            dvb = (
                dist_t[:]
                .rearrange("p (f n) -> p f n", f=F)
                .unsqueeze(3)
                .broadcast_to([P, F, N, C])
            )
            nc.vector.tensor_add(out4, in4, dvb)

            dst = bass.AP(
                yout,
                k * F * FW,
                [[S * FW, BC], [SH * FW, H], [1, F * FW]],
            )
            nc.gpsimd.dma_start(dst, out_t[:])

    nc.compile()
    return nc


def kernel(x: np.ndarray, **_unused) -> np.ndarray:
    x = np.ascontiguousarray(np.asarray(x), dtype=np.float32)
    assert x.shape == (B, S, N, C), x.shape

    if "nc" not in _cache:
        _cache["nc"] = _build()
    nc = _cache["nc"]

    in_maps = []
    for ci in range(NCORES):
        xc = x[ci * BC:(ci + 1) * BC].reshape(BC, S * FW)
        xp = np.empty((BC, (S + 1) * FW), dtype=np.float32)
        xp[:, : S * FW] = xc
        # pad frame = copy of the last frame -> diff at s = S-1 is 0
        xp[:, S * FW:] = xc[:, (S - 1) * FW:]
        in_maps.append({"xin": xp.reshape(IN_FLAT)})

    res = run_bass_kernel_spmd(nc, in_maps, core_ids=list(range(NCORES)))
    _cache["last_results"] = res

    out = np.empty((B, S, N, C), dtype=np.float32)
    for ci in range(NCORES):
        out[ci * BC:(ci + 1) * BC] = res.results[ci]["yout"].reshape(
            BC, S, N, C
        )
    return out


# revision 14
# speedup vs baseline: 1.2052x; 1.0548x over previous
"""Trainium2 Bass kernel for nn_PositionalEncoding_61151744360729.

out[b, s, n, :] = x[b, s, n, :] + ||x[b, s+1, n, :] - x[b, s, n, :]||_2
(with distance 0 at s = S-1).

Sharding: data-parallel on batch across 8 NeuronCores (64 batches/core).
On-core layout: partition p = b*2 + h (b = batch, h = sequence half),
free dim = frames*75 floats, so every DMA is a large contiguous span per
partition and the outermost AP dim (64) lets SWDGE fan descriptors over
all 16 SDMA engines. Each batch is padded host-side with a copy of its
last frame, which makes the last-frame distance exactly 0 with no
special-casing. Per 64-frame chunk: DVE shifted subtract -> ACT square
-> two strided DVE adds (sum over the 3 coords) -> ACT sqrt -> three
strided DVE broadcast-adds -> DMA out.
"""

import sys
from contextlib import ExitStack

for _p in ("/opt/trn_rl_repo", "/root/.axon_site/_ro/trn_rl_repo"):
    if _p not in sys.path:
        sys.path.insert(0, _p)

import numpy as np

import concourse.bass as bass
import concourse.tile as tile
from concourse import bacc, mybir
from concourse.bass_utils import run_bass_kernel_spmd

B, S, N, C = 512, 1024, 25, 3
FW = N * C                 # 75 floats per frame
NCORES = 8
BC = B // NCORES           # 64 batches per core
H = 2                      # sequence halves -> 128 partitions
SH = S // H                # 512 frames per half
P = H * BC                 # 128 partitions
F = 64                     # frames per chunk per partition
K = SH // F                # 8 chunks
IN_FLAT = BC * (S + 1) * FW   # input padded by one zero frame per batch
OUT_FLAT = BC * S * FW

_cache = {}


def _build():
    f32 = mybir.dt.float32
    Af = mybir.ActivationFunctionType
    nc = bacc.Bacc(
        "TRN2", target_bir_lowering=False, debug=False, num_devices=NCORES
    )
    xin = nc.dram_tensor("xin", [IN_FLAT], f32, kind="ExternalInput")
    yout = nc.dram_tensor("yout", [OUT_FLAT], f32, kind="ExternalOutput")

    with tile.TileContext(nc) as tc, ExitStack() as ctx:
        pin = ctx.enter_context(tc.tile_pool(name="pin", bufs=4))
        pmid = ctx.enter_context(tc.tile_pool(name="pmid", bufs=2))
        psm = ctx.enter_context(tc.tile_pool(name="psm", bufs=3))
        pout = ctx.enter_context(tc.tile_pool(name="pout", bufs=2))

        PF = 3  # input prefetch depth

        def issue_in(k):
            t = pin.tile([P, (F + 1) * FW], f32)
            src = bass.AP(
                xin,
                k * F * FW,
                [[(S + 1) * FW, BC], [SH * FW, H], [1, (F + 1) * FW]],
            )
            nc.gpsimd.dma_start(t[:], src)
            return t

        in_tiles = [issue_in(k) for k in range(PF)]

        for k in range(K):
            in_t = in_tiles[k]

            diff_t = pmid.tile([P, F * FW], f32)
            nc.vector.tensor_sub(
                diff_t[:], in_t[:, FW:(F + 1) * FW], in_t[:, 0:F * FW]
            )
            nc.scalar.activation(diff_t[:], diff_t[:], Af.Square)

            sq4 = diff_t[:].rearrange("p (f n c) -> p f n c", f=F, n=N, c=C)
            dist2_t = psm.tile([P, F * N], f32)
            d2 = dist2_t[:].rearrange("p (f n) -> p f n", f=F)
            nc.vector.tensor_add(d2, sq4[:, :, :, 0], sq4[:, :, :, 1])
            nc.vector.tensor_add(d2, d2, sq4[:, :, :, 2])
            # sqrt in place: dist2_t becomes dist
            nc.scalar.activation(dist2_t[:], dist2_t[:], Af.Sqrt)
            dist_t = dist2_t

            if k + PF < K:
                in_tiles.append(issue_in(k + PF))

            out_t = pout.tile([P, F * FW], f32)
            out4 = out_t[:].rearrange("p (f n c) -> p f n c", f=F, n=N, c=C)
            in4 = in_t[:, 0:F * FW].rearrange(
                "p (f n c) -> p f n c", f=F, n=N, c=C
            )
            dvb = (
                dist_t[:]
                .rearrange("p (f n) -> p f n", f=F)
                .unsqueeze(3)
                .broadcast_to([P, F, N, C])
            )
            nc.vector.tensor_add(out4, in4, dvb)

            dst = bass.AP(
                yout,
                k * F * FW,
                [[S * FW, BC], [SH * FW, H], [1, F * FW]],
            )
            nc.gpsimd.dma_start(dst, out_t[:])

    nc.compile()
    return nc


def kernel(x: np.ndarray, **_unused) -> np.ndarray:
    x = np.ascontiguousarray(np.asarray(x), dtype=np.float32)
    assert x.shape == (B, S, N, C), x.shape

    if "nc" not in _cache:
        _cache["nc"] = _build()
    nc = _cache["nc"]

    in_maps = []
    for ci in range(NCORES):
        xc = x[ci * BC:(ci + 1) * BC].reshape(BC, S * FW)
        xp = np.empty((BC, (S + 1) * FW), dtype=np.float32)
        xp[:, : S * FW] = xc
        # pad frame = copy of the last frame -> diff at s = S-1 is 0
        xp[:, S * FW:] = xc[:, (S - 1) * FW:]
        in_maps.append({"xin": xp.reshape(IN_FLAT)})

    res = run_bass_kernel_spmd(nc, in_maps, core_ids=list(range(NCORES)))
    _cache["last_results"] = res

    out = np.empty((B, S, N, C), dtype=np.float32)
    for ci in range(NCORES):
        out[ci * BC:(ci + 1) * BC] = res.results[ci]["yout"].reshape(
            BC, S, N, C
        )
    return out
